# revision 3
# baseline (speedup 1.0000x reference)
"""Trainium2 Bass kernel for nn_AuxiliaryClustering (segment_reduce), v2.

Data-parallel over batch on 8 cores; each core streams its 125k-row
shard once (a in bf16, z in fp8-e4m3). vs v1: a is bf16 (half the DMA),
the s=a-max pass is gone (GpSimd compares a==rowmax directly; host
pre-bumps bf16 ties by 1 ulp so the rowmax is unique), colsum runs on
raw a (no maxsum correction), and the gather is flipped: z is PE-
transposed into PSUM ([d, row] layout), a constant block-diag -C
stationary gathers centers on top, so the d^2 row-sum becomes 8 tiny
PE matmuls against a ones-vector instead of a DVE reduce.

Per 2048-row tile:
  DVE    rowmax (2 halves), ohT PSUM->SBUF copy
  GpSimd oh = is_equal(a, rowmax bcast) -> bf16 one-hot
  PE     8 z-chunk transposes (zT into PSUM), 8 oh transposes,
         2 gather matmuls (const Cbd stationary) -> diffT,
         8 colsum (a-chunk stationary x ones),
         8 d^2-sum (dsqT-chunk stationary x ones2) -> [p, j, e],
         8 papp (oh-chunk stationary x [dist0, dist1, 1])
  ACT    square diffT->SBUF bf16, sqrt d^2 -> dist
Whole-kernel papp/colsum accumulate in PSUM via start/stop chains.
"""

import os
from contextlib import ExitStack

import ml_dtypes
import numpy as np

import concourse.bass as bass
import concourse.bacc as bacc
import concourse.tile as tile
from concourse import mybir
from concourse.bass_utils import run_bass_kernel_spmd

F32 = mybir.dt.float32
FP8 = mybir.dt.float8e4
BF16 = mybir.dt.bfloat16
AX = mybir.AxisListType
OP = mybir.AluOpType

B, D, K = 1000000, 64, 64
NCORES = 8
P = 128
R = 16
TILE_ROWS = P * R                                   # 2048
SHARD = B // NCORES                                 # 125000
NTILES = (SHARD + TILE_ROWS - 1) // TILE_ROWS       # 62
PAD_SHARD = NTILES * TILE_ROWS                      # 126976
NPAD = PAD_SHARD - SHARD                            # 1976
NCHUNK = R * D // P                                 # 8 chunks (r-pairs)
HR = R // 2

EPS = 1e-08
WEIGHT = 0.1


def build_nc(ntiles: int = NTILES, pad_shard: int = PAD_SHARD):
    nc = bacc.Bacc("TRN2", target_bir_lowering=False, debug=False)

    a_d = nc.dram_tensor("a", [pad_shard, K], BF16, kind="ExternalInput").ap()
    # host-pretransposed z: row (e*64+d), col (t, j, p) for rows (p, 2j+e)
    zt_d = nc.dram_tensor("zt", [P, pad_shard // 2], FP8, kind="ExternalInput").ap()
    # block-diagonal [[-C,0],[0,-C]] bf16: rows (e k), cols (e d)
    cbd_d = nc.dram_tensor("cbd", [P, P], BF16, kind="ExternalInput").ap()
    ident_d = nc.dram_tensor("ident", [P, P], BF16, kind="ExternalInput").ap()
    ident8_d = nc.dram_tensor("ident8", [P, P], FP8, kind="ExternalInput").ap()

    # out cols: 0:3 papp [dist_e0, dist_e1, count]; 3 colsum
    outk_d = nc.dram_tensor("out_k", [P, 8], F32, kind="ExternalOutput").ap()

    a_4d = a_d.rearrange("(t p r) d -> t p r d", p=P, r=R)
    zt_3d = zt_d.rearrange("q (t n) -> t q n", n=TILE_ROWS // 2)

    with tile.TileContext(nc) as tc, ExitStack() as ctx:
        iop = ctx.enter_context(tc.tile_pool(name="io", bufs=8))
        wp = ctx.enter_context(tc.tile_pool(name="work", bufs=8))
        cp = ctx.enter_context(tc.tile_pool(name="const", bufs=1))
        ps_diff = ctx.enter_context(tc.tile_pool(name="ps_diff", bufs=2, space="PSUM"))
        ps_oh = ctx.enter_context(tc.tile_pool(name="ps_oh", bufs=2, space="PSUM"))
        ps_acc = ctx.enter_context(tc.tile_pool(name="ps_acc", bufs=1, space="PSUM"))
        ps_dd = ctx.enter_context(tc.tile_pool(name="ps_dd", bufs=1, space="PSUM"))

        # --- constants ---
        ident8_t = cp.tile([P, P], FP8)
        nc.sync.dma_start(out=ident8_t[:], in_=ident8_d[:])
        cbd_t = cp.tile([P, P], BF16)
        nc.sync.dma_start(out=cbd_t[:], in_=cbd_d[:])
        ident_t = cp.tile([P, P], BF16)
        nc.sync.dma_start(out=ident_t[:], in_=ident_d[:])
        ones_bf = cp.tile([P, 1], BF16)
        nc.vector.memset(ones_bf[:], 1.0)
        # ones2[(e d), n] = 1 iff e == n  (d^2 row-sum selector)
        ones2_t = cp.tile([P, 2], BF16)
        nc.vector.memset(ones2_t[:], 0.0)
        nc.vector.memset(ones2_t[0:64, 0:1], 1.0)
        nc.vector.memset(ones2_t[64:128, 1:2], 1.0)
        zeros_t = cp.tile([P, 4], BF16)
        nc.vector.memset(zeros_t[:], 0.0)

        state = {}

        def dma_in(i):
            a_t = iop.tile([P, R, K], BF16, tag="a")
            nc.sync.dma_start(out=a_t[:], in_=a_4d[i])
            z_t = iop.tile([P, TILE_ROWS // 2], FP8, tag="z")
            nc.sync.dma_start(out=z_t[:], in_=zt_3d[i])
            state[i] = dict(a=a_t, z=z_t)

        def maxsub(i):
            """rowmax (DVE, negated) + s = a - max (GpSimd broadcast add).
            bf16 s keeps the sign exactly: the max element gives 0."""
            st = state[i]
            a_t = st["a"]
            negm_t = wp.tile([P, R, 1], F32, tag="mx")
            s_t = wp.tile([P, R, K], BF16, tag="s")
            for h in range(2):
                rs = slice(h * HR, (h + 1) * HR)
                nc.vector.reduce_max(negm_t[:, rs, :], a_t[:, rs, :],
                                     axis=AX.X, negate=True)
                nc.gpsimd.tensor_tensor(
                    out=s_t[:, rs, :], in0=a_t[:, rs, :],
                    in1=negm_t[:, rs, :].broadcast_to([P, HR, K]),
                    op=OP.add,
                )
            st["s"] = s_t

        def iseq(i):
            """one-hot = (s == 0): DVE tensor_scalar, 4x bf16 tier on flat
            2D halves. Issued a full iteration after the GpSimd subs so
            the DVE never stalls on the Pool semaphore."""
            st = state[i]
            oh_t = wp.tile([P, R, K], BF16, tag="oh")
            s2d = st["s"][:].rearrange("p r d -> p (r d)")
            oh2d = oh_t[:].rearrange("p r d -> p (r d)")
            half = R * K // 2
            for h in range(2):
                cs = slice(h * half, (h + 1) * half)
                nc.vector.tensor_scalar(
                    out=oh2d[:, cs], in0=s2d[:, cs],
                    scalar1=0.0, scalar2=None, op0=OP.is_equal,
                )
            st["oh"] = oh_t

        def front_b(i):
            """PE transposes (z + oh), ohT copy, colsum."""
            st = state[i]
            oh2d = st["oh"][:].rearrange("p r d -> p (r d)")
            a2d = st["a"][:].rearrange("p r d -> p (r d)")

            # zT -> diffT PSUM [ (e d), (j p) ] f32 via fp8 identity matmul
            diff_ps = ps_diff.tile([P, 2, NCHUNK // 2 * P], F32, tag="diffT")
            half = NCHUNK * P // 2
            for h in range(2):
                nc.tensor.matmul(
                    diff_ps[:, h, :],
                    ident8_t[:],
                    st["z"][:, h * half:(h + 1) * half],
                    start=True, stop=False,
                    skip_group_check=True,
                )
            # oh transposes -> ohT PSUM bf16; copy to SBUF per half so
            # the h0 gather's input is ready ~400ns earlier (the gather
            # was measured stalling ~350ns on the whole-tile copy)
            ohT_ps = ps_oh.tile([P, NCHUNK, P], BF16, tag="ohTp")
            ohT_t = wp.tile([P, NCHUNK, P], BF16, tag="ohT")
            for h in range(2):
                js = slice(h * (NCHUNK // 2), (h + 1) * (NCHUNK // 2))
                for j in range(h * (NCHUNK // 2), (h + 1) * (NCHUNK // 2)):
                    nc.tensor.transpose(
                        out=ohT_ps[:, j, :],
                        in_=oh2d[:, j * P:(j + 1) * P],
                        identity=ident_t[:],
                    )
                nc.vector.tensor_copy(out=ohT_t[:, js, :], in_=ohT_ps[:, js, :])

            # colsum of raw a: a-chunk stationary x ones
            for j in range(NCHUNK):
                nc.tensor.matmul(
                    colsum_ap,
                    a2d[:, j * P:(j + 1) * P],
                    ones_bf[:],
                    start=(i == 0 and j == 0),
                    stop=(i == ntiles - 1 and j == NCHUNK - 1),
                )
            st["diff"] = diff_ps
            st["ohT"] = ohT_t

        def mid(i):
            """Gather centers onto zT, square."""
            st = state[i]
            ohT2d = st["ohT"][:].rearrange("p c q -> p (c q)")
            half = NCHUNK * P // 2
            for h in range(2):
                nc.tensor.matmul(
                    st["diff"][:, h, :],
                    cbd_t[:],
                    ohT2d[:, h * half:(h + 1) * half],
                    start=False, stop=True,
                    skip_group_check=True,
                )
            dsq_t = wp.tile([P, NCHUNK, P], BF16, tag="dsq")
            nc.scalar.square(dsq_t[:].rearrange("p c q -> p (c q)"),
                             st["diff"][:].rearrange("p h q -> p (h q)"))
            st["dsq"] = dsq_t

        def back1(i):
            """d^2 row-sums on PE, sqrt into papp-rhs."""
            st = state[i]
            for j in range(NCHUNK):
                nc.tensor.matmul(
                    dd_ap[:, j, :],
                    st["dsq"][:, j, :],
                    ones2_t[:],
                    start=True, stop=True,
                )
            rhs_t = wp.tile([P, NCHUNK, 3], BF16, tag="rhs")
            if i < 8:  # pool rotates 8 bufs; ones column persists after
                nc.vector.memset(rhs_t[:, :, 2:3], 1.0)
            nc.scalar.sqrt(rhs_t[:, :, 0:2], dd_ap[:])
            st["rhs"] = rhs_t

        def back2(i):
            """per-cluster [dist_e0, dist_e1, count] partials."""
            st = state.pop(i)
            oh2d = st["oh"][:].rearrange("p r d -> p (r d)")
            for j in range(NCHUNK):
                nc.tensor.matmul(
                    papp_ap,
                    oh2d[:, j * P:(j + 1) * P],
                    st["rhs"][:, j, :],
                    start=False,
                    stop=(i == ntiles - 1 and j == NCHUNK - 1),
                    skip_group_check=True,
                )

        def front_a(i):
            dma_in(i)
            maxsub(i)
            iseq(i)

        for i in range(min(3, ntiles)):
            front_a(i)

        # accumulators get a bank with no start=True traffic (a start=True
        # write resets has_written of other partial regions in its bank);
        # the per-tile dd writes live in their own bank
        acc_ps = ps_acc.tile([P, 4], F32, tag="acc")
        papp_ap = acc_ps[:, 0:3]
        colsum_ap = acc_ps[:, 3:4]
        dd_t = ps_dd.tile([P, 512], F32, tag="dd")
        dd_ap = dd_t[:, 0:16].rearrange("p (j e) -> p j e", e=2)



        # prime the papp region with start=True zeros BEFORE any colsum
        # accumulation: a later start=True in this bank would reset the
        # has_written bits of the colsum chain (cost us tiles 0-3 once)
        nc.tensor.matmul(papp_ap, ident_t[:], zeros_t[:, 0:3],
                         start=True, stop=False, skip_group_check=True)

        front_b(0)
        for i in range(ntiles):
            if i + 3 < ntiles:
                front_a(i + 3)
            mid(i)
            if i + 1 < ntiles:
                front_b(i + 1)
            if i >= 1:
                back1(i - 1)
            if i >= 2:
                back2(i - 2)
        back1(ntiles - 1)
        back2(ntiles - 2)
        back2(ntiles - 1)

        resk_t = cp.tile([P, 8], F32)
        nc.vector.memset(resk_t[:], 0.0)
        nc.vector.tensor_copy(out=resk_t[:, 0:4], in_=acc_ps[:])
        nc.sync.dma_start(out=outk_d[:], in_=resk_t[:])

    nc.finalize()
    return nc


_NC_CACHE = {}


def _get_nc():
    if "nc" not in _NC_CACHE:
        _NC_CACHE["nc"] = build_nc()
    return _NC_CACHE["nc"]


def _prep_a_bf16(a):
    """Cast to bf16 and bump first-argmax of tied rows by 1 ulp so the
    bf16 rowmax is unique (multi-hot rows would corrupt the gather)."""
    ab = a.astype(ml_dtypes.bfloat16)
    m = ab.max(axis=1, keepdims=True)
    ties = (ab == m).sum(axis=1) > 1
    rows = np.nonzero(ties)[0]
    if rows.size:
        cols = np.argmax(ab[rows] == m[rows], axis=1)
        u = ab.view(np.uint16)
        u[rows, cols] += 1  # +1 ulp: strictly exceeds old max, stays < 1.01
    return ab


def kernel(latent_z, cluster_assignments, cluster_centers):
    z = np.asarray(latent_z, dtype=np.float32)
    a = np.ascontiguousarray(np.asarray(cluster_assignments, dtype=np.float32))
    c = np.ascontiguousarray(np.asarray(cluster_centers, dtype=np.float32))

    ab = _prep_a_bf16(a)
    zb = z.astype(ml_dtypes.float8_e4m3fn)

    cbf = c.astype(ml_dtypes.bfloat16)
    cbd = np.zeros((P, P), dtype=ml_dtypes.bfloat16)
    cbd[:K, :D] = -cbf
    cbd[K:, D:] = -cbf

    # pad rows: a = e0 (unique max at k=0), z = fp8(c0) so dist ~ 0
    a_pad_row = np.zeros((K,), dtype=ml_dtypes.bfloat16)
    a_pad_row[0] = 1.0
    z_pad_row = c[0].astype(ml_dtypes.float8_e4m3fn)

    in_maps = []
    for core in range(NCORES):
        lo, hi = core * SHARD, (core + 1) * SHARD
        a_s = np.empty((PAD_SHARD, K), dtype=ml_dtypes.bfloat16)
        z_s = np.empty((PAD_SHARD, D), dtype=ml_dtypes.float8_e4m3fn)
        a_s[:SHARD] = ab[lo:hi]
        z_s[:SHARD] = zb[lo:hi]
        a_s[SHARD:] = a_pad_row
        z_s[SHARD:] = z_pad_row
        # zT packing: [e, d, t, j, p] <- z[(t, p, 2j+e), d]
        zt = z_s.reshape(NTILES, P, NCHUNK, 2, D).transpose(3, 4, 0, 2, 1)
        zt = np.ascontiguousarray(zt).reshape(P, PAD_SHARD // 2)
        in_maps.append({
            "a": a_s, "zt": zt, "cbd": cbd,
            "ident": np.eye(P, dtype=np.float32).astype(ml_dtypes.bfloat16),
            "ident8": np.eye(P, dtype=np.float32).astype(ml_dtypes.float8_e4m3fn),
        })

    nc = _get_nc()
    trace = bool(int(os.environ.get("KERNEL_PROFILE", "0")))
    res = run_bass_kernel_spmd(
        nc, in_maps, list(range(NCORES)), trace=trace, trace_cores=[0],
    )
    if trace:
        _NC_CACHE["exec_time_ns"] = res.exec_time_ns
        print(f"HW exec time: {res.exec_time_ns} ns")

    # ---- host-side all-reduce of [K]-partials + final scalar math ----
    outk = np.stack([r["out_k"] for r in res.results])        # [8, P, 8]

    dist_sum = (outk[:, :K, 0] + outk[:, K:, 1]).sum(axis=0).astype(np.float64)
    counts = (outk[:, :K, 2] + outk[:, K:, 2]).sum(axis=0).astype(np.float64)
    colsum = (outk[:, :K, 3] + outk[:, K:, 3]).sum(axis=0).astype(np.float64)

    # remove padding contributions (pad rows all land in cluster 0)
    counts[0] -= NCORES * NPAD
    colsum[0] -= NCORES * NPAD

    cd = c.astype(np.float64)
    dif = cd[:, None, :] - cd[None, :, :]
    cdist = np.sqrt(np.maximum((dif * dif).sum(-1), 0.0))
    separation = float(-cdist.sum() / (K * (K - 1)))

    probs = colsum / B
    balance = float(np.sum((1.0 / K) * (np.log(1.0 / K) - np.log(probs + EPS))))
    nonempty = counts > 0
    per_mean = dist_sum / np.maximum(counts, 1.0)
    n_nonempty = float(nonempty.sum())
    compact = float(np.sum(np.where(nonempty, per_mean, 0.0)) / max(n_nonempty, 1.0))
    aux = WEIGHT * balance + WEIGHT * separation + WEIGHT * compact
    cluster_balance = float(np.std(probs, ddof=1))

    return (
        np.float32(aux),
        np.float32(balance),
        np.float32(separation),
        np.float32(compact),
        np.float32(cluster_balance),
    )
